# revision 53
# baseline (speedup 1.0000x reference)
"""Causal self-attention block (qkv proj + 16-head causal attention + out_proj
folded with c_proj) on 8 trn2 NeuronCores, data-parallel over the batch (B=8:
one batch element per core).

Layout strategy (per core, batch element b):
  - Activations are kept feature-major [feature, token] on chip so every
    linear layer is a plain   out = W_T.T @ act   matmul chain with the
    (host-pre-transposed) weight as the stationary operand. No on-device
    transposes at all.
  - out_proj and c_proj are two back-to-back linears with no nonlinearity, so
    they are folded on the host into one matmul: W = (Wc Wo)^T, b = Wc bo+bc.
  - Attention computes transposed scores  sT[tk, tq] = k_h.T q_h  per head
    pair (row-tiled K=64 matmuls), exp with no max-subtraction (scores here
    are bounded by a few units), causal mask applied to the exp'd diagonal
    block on DVE, and the AV product consumes sT directly with token-major V
    tiles as the stationary operand. A fused ones-row in the V operand (M=65)
    yields the softmax denominator for free. Normalization is pipelined
    per head-pair: reciprocal_approx_fast straight off the psum denominator
    row, a DMA bounce through DRAM to broadcast 1/denom across partitions,
    and an in-place multiply — so the final out_proj chunk starts ~3us after
    the last AV matmul instead of waiting on a batched normalize.
  - All matmuls run in float32r (TF32-like, ~1e-4 rel precision, 4x the
    throughput of fp32 on the PE).
  - Phases are software-pipelined so the in-order PE queue always has dense
    matmul work while ACT paces the attention exps:
      S1 : qk features pairs 0-3 + V heads 0-7 (x/w DMAs interleaved;
           consts deferred; ACT exp table preloaded via a warmup exp)
      S2a: attention pairs 0-3 (both chunks) over qk pairs 4-7 + V heads
           8-15 (token tiles 0-3)
      S2b: attention chunk-0 pairs 4-7 over V heads 8-15 (token tiles 4-7)
      S3 : attention chunk-1 pairs 4-7 over out_proj chunk 0
      S4 : out_proj chunk 1 (4-deep psum pipeline)
"""

import sys

if "/opt/trn_rl_repo" not in sys.path:
    sys.path.insert(0, "/opt/trn_rl_repo")

import ml_dtypes
import numpy as np

import concourse.bass as bass
import concourse.tile as tile
from concourse import bacc, mybir
from concourse.bass_utils import run_bass_kernel_spmd

B, T, E, H = 8, 1024, 1024, 16
DH = E // H          # 64
JQK = 2 * E          # q+k fused feature dim (2048)
F32 = mybir.dt.float32
F32R = mybir.dt.float32r
BF16 = mybir.dt.bfloat16
Act = mybir.ActivationFunctionType

TRACE = False        # test harness flips this for profiled runs
PHASE_LIMIT = 4      # debug: 1=qk proj, 2=+v, 3=+attention, 4=full
_CACHE = {}


def _emit(nc, tc, aps):
    (xT, wqkT, wvT, bqk, bvrow, woutT, bout, mask01, ones,
     onesbf, ind33, outT) = aps
    ET = E // 128     # 8  e-tiles (contraction)
    TT = T // 128     # 8  token tiles
    NT = T // 512     # 2  512-wide token column chunks

    consts = tc.alloc_tile_pool(name="consts", bufs=1)
    onesb = consts.tile([128, 128], F32R, tag="onesb")
    mask01b = consts.tile([128, 128], BF16, tag="mask01b")
    bqkb = consts.tile([128, JQK // 128], F32, tag="bqkb")
    bvb = consts.tile([1, E], F32R, tag="bvb")
    boutb = consts.tile([128, E // 128], F32, tag="boutb")
    ind33b = consts.tile([33, 128], F32R, tag="ind33b")
    warm = consts.tile([1, 8], BF16, tag="warm")

    def issue_consts():
        # Deferred until the first w/x DMAs are in flight so the PE can start
        # ~4us earlier; bqkb leads because the first qk IDENTITY needs it.
        nc.sync.dma_start(out=bqkb, in_=bqk)
        nc.gpsimd.dma_start(out=mask01b, in_=mask01)
        nc.gpsimd.dma_start(out=onesb, in_=ones)
        nc.gpsimd.dma_start(out=bvb[0:1, :], in_=bvrow)
        nc.gpsimd.dma_start(out=boutb, in_=bout)
        nc.gpsimd.dma_start(out=ind33b, in_=ind33)
        # Warmup exp: pulls the ACT exp-table load off the first real
        # attention exp's critical path (fires during dense S1).
        nc.scalar.activation(out=warm, in_=mask01b[0:1, 0:8], func=Act.Exp,
                             scale=1.0 / 8.0)

    psum_mm = tc.alloc_tile_pool(name="psum_mm", bufs=1, space="PSUM")
    psum_att = tc.alloc_tile_pool(name="psum_att", bufs=1, space="PSUM")
    p_dram = tc.alloc_tile_pool(name="p_dram", bufs=1, space="DRAM")
    p_y = tc.alloc_tile_pool(name="p_y", bufs=1)
    p_qk = tc.alloc_tile_pool(name="p_qk", bufs=1)
    p_v = tc.alloc_tile_pool(name="p_v", bufs=1)
    p_w3 = tc.alloc_tile_pool(name="p_w3", bufs=16)
    p_x = tc.alloc_tile_pool(name="p_x", bufs=1)
    p_wqk = tc.alloc_tile_pool(name="p_wqk", bufs=16)
    dden = p_dram.tile([64, 512], F32, tag="dden")
    yt = p_y.tile([128, ET, T], F32R)
    qkt = p_qk.tile([128, JQK // 128, T], F32R)
    vt = p_v.tile([128, TT, H, DH + 1], BF16)
    xt = p_x.tile([128, ET, T], BF16)

    def mm_psum(tag="mm", pool=None, bufs=2):
        return (pool or psum_mm).tile([128, 512], F32, tag=tag, bufs=bufs,
                                      name="ps_" + tag)

    # ---- dense generators: qkv projection ---------------------------------
    def qk_gen(jg, after=None):
        """qkT[j, t] = Wqk x^T + bqk for the 512-wide feature group jg."""
        wtiles = []
        for et in range(ET):
            wt = p_wqk.tile([128, 512], BF16, tag="wqk", name="wt")
            # first weight group split across the sync and gpsimd rings so
            # the DMA-latency-bound startup streams two tiles at a time
            ring = nc.gpsimd if (jg == 0 and et % 2 == 1) else nc.sync
            ring.dma_start(out=wt, in_=wqkT[et * 128:(et + 1) * 128,
                                            jg * 512:(jg + 1) * 512])
            wtiles.append(wt)
            if jg == 0:                    # x loads ride the ACT DGE ring —
                nc.scalar.dma_start(       # startup streams w (sync) and x
                    out=xt[:, et, 0:512],  # (scalar) in parallel
                    in_=xT[et * 128:(et + 1) * 128, 0:512])
        if after is not None:
            after()
        if jg == 0:
            for et in range(ET):
                nc.gpsimd.dma_start(out=xt[:, et, 512:1024],
                                    in_=xT[et * 128:(et + 1) * 128,
                                           512:1024])
        for th in range(NT):
            for js in range(4):
                jt = jg * 4 + js
                ps = mm_psum()
                for et in range(ET):
                    nc.tensor.matmul(
                        ps,
                        wtiles[et][:, js * 128:(js + 1) * 128],
                        xt[:, et, th * 512:(th + 1) * 512],
                        start=(et == 0), stop=(et == ET - 1))
                    yield
                nc.scalar.activation(
                    out=qkt[:, jt, th * 512:(th + 1) * 512], in_=ps,
                    func=Act.Identity, bias=bqkb[:, jt:jt + 1], scale=1.0)

    wv_cache = {}

    def vb_gen(jh, tt0, tt1):
        """v[t, h, d] token-major for heads 8*jh..8*jh+7 (+bias via ones-row
        matmul), with a bf16 ones column at d=64 for the fused denominator.
        Emits token tiles tt0..tt1 (split so S2b keeps dense filler)."""
        if jh == 0 and tt0 == 0:
            for tt in range(TT):
                nc.gpsimd.dma_start(out=vt[:, tt, :, DH], in_=onesbf)
        if jh in wv_cache:
            wvtiles = wv_cache[jh]
        else:
            wvtiles = []
            for et in range(ET):
                wt = p_wqk.tile([128, 512], BF16, tag="wqk", name="wt")
                nc.sync.dma_start(out=wt, in_=wvT[et * 128:(et + 1) * 128,
                                                 jh * 512:(jh + 1) * 512])
                wvtiles.append(wt)
            wv_cache[jh] = wvtiles
        for tt in range(tt0, tt1):
            ps = mm_psum()
            for et in range(ET):
                nc.tensor.matmul(
                    ps,
                    xt[:, et, tt * 128:(tt + 1) * 128],
                    wvtiles[et],
                    start=(et == 0), stop=False)
                yield
            nc.tensor.matmul(
                ps, onesb[0:1, 0:128], bvb[0:1, jh * 512:(jh + 1) * 512],
                start=False, stop=True)
            yield
            # the last V drains land amid the S2b drain chains on DVE; route
            # them to ACT (Identity, NOT Copy — Copy swaps the act table and
            # forces a 1.3us reload around every exp)
            if jh == 1 and tt >= 6:
                nc.scalar.activation(
                    out=vt[:, tt, jh * 8:(jh + 1) * 8, 0:DH],
                    in_=ps.rearrange("p (h d) -> p h d", d=DH),
                    func=Act.Identity, bias=0.0, scale=1.0)
            else:
                nc.vector.tensor_copy(
                    out=vt[:, tt, jh * 8:(jh + 1) * 8, 0:DH],
                    in_=ps.rearrange("p (h d) -> p h d", d=DH))

    # ---- attention generator (yields once per tk-iteration) ---------------
    LAG = 3

    def att_gen(c, a, p_esc, p_nrm):
        cs = c * 512
        last_it = 4 * c + 3
        qj = a                             # q tile of the pair
        kj = (JQK // 2) // 128 + a         # k tile of the pair
        avps = [psum_att.tile([128, 512], F32, tag=f"av{p}", bufs=1,
                              name=f"avp{p}") for p in range(2)]
        pend = []

        def emit_av(it, sub, clen, esc):
            for p in range(2):
                nc.tensor.matmul(
                    avps[p][0:DH + 1, sub:sub + clen],
                    vt[:, it, 2 * a + p, :],
                    esc[:, p, :clen],
                    start=(it == 0), stop=(it == last_it),
                    skip_group_check=True)

        for it in range(last_it + 1):
            n0 = it * 128
            lo = max(n0, cs)
            sub = lo - cs
            clen = 512 - sub
            scp = psum_att.tile([128, 2, 512], F32, tag="sc", bufs=2,
                                name="scp")
            for p in range(2):             # paired heads: row-tiled matmuls
                pb = p * 64
                nc.tensor.matmul(
                    scp[:, p, :clen],
                    qkt[pb:pb + 64, kj, n0:n0 + 128],
                    qkt[pb:pb + 64, qj, lo:lo + clen],
                    start=True, stop=True)
            esc = p_esc.tile([128, 2, 512], BF16, tag="esc", name="esc")
            nc.scalar.activation(out=esc[:, :, :clen], in_=scp[:, :, :clen],
                                 func=Act.Exp, scale=1.0 / 8.0)
            if n0 >= cs:                   # diagonal block: causal mask,
                nc.vector.tensor_mul(      # off the PE chain thanks to LAG
                    esc[:, :, 0:128], esc[:, :, 0:128],
                    mask01b[:, None, :].broadcast_to([128, 2, 128]))
            pend.append((it, sub, clen, esc))
            if len(pend) > LAG:
                emit_av(*pend.pop(0))
            yield
        for args in pend:
            emit_av(*args)
        for p in range(2):                 # drain unnormalized y
            nc.vector.tensor_copy(out=yt[p * 64:p * 64 + 64, qj,
                                         cs:cs + 512],
                                  in_=avps[p][0:DH, :])
        # Per-pair normalization, pipelined behind the remaining attention
        # units: reciprocal straight off the psum denominator rows (partition
        # bases 64/96 are engine-addressable), one DMA out to DRAM, a
        # partition-broadcast DMA back, and an in-place multiply.
        # custom-DVE ops only honor partition offset 0, so stage the den rows
        # to partitions 0/32 of a scratch tile and recip in place.
        rst = [p_nrm.tile([128, 512], F32, tag="rstg", bufs=4, name="rstg")
               for _ in range(2)]
        r = 32 * c + 2 * a
        for p in range(2):
            nc.vector.tensor_copy(out=rst[p][0:1, :],
                                  in_=avps[p][DH:DH + 1, :])
            nc.vector.reciprocal_approx_fast(out=rst[p][0:1, :],
                                             in_=rst[p][0:1, :])
        for p in range(2):
            nc.sync.dma_start(out=dden[r + p:r + p + 1, :],
                              in_=rst[p][0:1, :])
        rb = p_nrm.tile([128, 512], F32, tag="rb", bufs=2, name="rb")
        for p in range(2):
            row = dden[r + p:r + p + 1, :]
            srcb = bass.AP(tensor=row.tensor, offset=row.offset,
                           ap=[[0, 64]] + list(row.ap)[1:])
            nc.sync.dma_start(out=rb[p * 64:(p + 1) * 64, :], in_=srcb)
        nc.vector.tensor_mul(yt[:, a, cs:cs + 512], yt[:, a, cs:cs + 512],
                             rb)

    # ---- drivers ----------------------------------------------------------
    def run_dense(dense, n=None):
        steps = 0
        while dense and (n is None or steps < n):
            try:
                next(dense[0])
                steps += 1
            except StopIteration:
                dense.pop(0)
        return steps

    def drive(att_units, dense, ratio=5):
        att_units = list(att_units)
        while att_units:
            try:
                next(att_units[0])
            except StopIteration:
                att_units.pop(0)
                continue
            run_dense(dense, ratio)
        run_dense(dense)

    # S1: dense deps for attention pairs 0-3
    dense1 = [qk_gen(0, after=issue_consts), qk_gen(2)]
    if PHASE_LIMIT >= 2:
        dense1.append(vb_gen(0, 0, TT))
    run_dense(dense1)

    # S2a: attention pairs 0-3 (both chunks) over qk pairs 4-7 + V tt 0-3
    p_esc1 = tc.alloc_tile_pool(name="p_esc1", bufs=4)
    p_nrm1 = tc.alloc_tile_pool(name="p_nrm1", bufs=1)
    dense2 = [qk_gen(1), qk_gen(3)]
    if PHASE_LIMIT >= 2:
        dense2.append(vb_gen(1, 0, 4))
    att2 = [att_gen(c, a, p_esc1, p_nrm1)
            for a in range(4) for c in range(NT)] if PHASE_LIMIT >= 3 else []
    drive(att2, dense2, ratio=4)

    # S2b: attention chunk-0 pairs 4-7 and chunk-1 pairs 4-5 interleaved over
    # the remaining V token tiles. The chunk-1 units give the PE independent
    # score/exp work while the chunk-0 drain + normalize chains serialize on
    # DVE and the DMA queue (the pipeline tail would otherwise idle ~10us).
    # prefetch the out_proj weights now — their DMAs have no deps and would
    # otherwise monopolize the queue exactly when S3's first groups need them
    wout_tiles = []
    if PHASE_LIMIT >= 4:
        for og in range(2):
            for et in range(ET):
                wt = p_w3.tile([128, 512], F32R, tag="w3", name="wt3")
                nc.sync.dma_start(
                    out=wt, in_=woutT[et * 128:(et + 1) * 128,
                                      og * 512:(og + 1) * 512])
                wout_tiles.append(wt)
    dense2b = [vb_gen(1, 4, TT)] if PHASE_LIMIT >= 2 else []
    att2b = [att_gen(c, a, p_esc1, p_nrm1)
             for c, a in [(0, 4), (0, 5), (1, 4), (0, 6), (1, 5), (0, 7)]
             ] if PHASE_LIMIT >= 3 else []
    drive(att2b, dense2b, ratio=3)
    p_nrm1.release()
    p_esc1.release()
    p_wqk.release()
    p_x.release()

    # S3: attention chunk-1 pairs 6-7 over out_proj chunk 0
    p_out = tc.alloc_tile_pool(name="p_out", bufs=3)
    p_esc2 = tc.alloc_tile_pool(name="p_esc2", bufs=4)
    p_nrm2 = tc.alloc_tile_pool(name="p_nrm2", bufs=1)

    def oproj_gen(th, pool, tag, bufs):
        for og in range(2):
            for os_ in range(4):
                ot = og * 4 + os_
                ps = mm_psum(tag=tag, pool=pool, bufs=bufs)
                for et in range(ET):
                    nc.tensor.matmul(
                        ps,
                        wout_tiles[og * ET + et][:, os_ * 128:(os_ + 1) * 128],
                        yt[:, et, th * 512:(th + 1) * 512],
                        start=(et == 0), stop=(et == ET - 1))
                    yield
                ob = p_out.tile([128, 512], F32, tag="ob", name="ob")
                nc.scalar.activation(out=ob, in_=ps, func=Act.Identity,
                                     bias=boutb[:, ot:ot + 1], scale=1.0)
                nc.sync.dma_start(
                    out=outT[ot * 128:(ot + 1) * 128,
                             th * 512:(th + 1) * 512],
                    in_=ob)

    att3 = [att_gen(1, a, p_esc2, p_nrm2)
            for a in range(6, 8)] if PHASE_LIMIT >= 3 else []
    dense3 = [oproj_gen(0, psum_mm, "mm", 2)] if PHASE_LIMIT >= 4 else []
    drive(att3, dense3, ratio=3)

    # S4: out_proj chunk 1 on a 4-deep psum pipeline (attention psums freed)
    psum_att.release()
    if PHASE_LIMIT >= 4:
        psum_tail = tc.alloc_tile_pool(name="psum_tail", bufs=1, space="PSUM")
        run_dense([oproj_gen(1, psum_tail, "mmt", 4)])
        psum_tail.release()
    p_nrm2.release()
    p_esc2.release()
    p_out.release()
    p_w3.release()
    p_v.release()
    p_qk.release()
    p_y.release()
    p_dram.release()
    psum_mm.release()
    consts.release()


def _build():
    if "nc" in _CACHE:
        return _CACHE["nc"]
    nc = bacc.Bacc("TRN2", target_bir_lowering=False, debug=False,
                   enable_asserts=True, num_devices=8)
    d = nc.dram_tensor
    aps = [
        d("xT", [E, T], BF16, kind="ExternalInput").ap(),
        d("wqkT", [E, JQK], BF16, kind="ExternalInput").ap(),
        d("wvT", [E, E], BF16, kind="ExternalInput").ap(),
        d("bqk", [128, JQK // 128], F32, kind="ExternalInput").ap(),
        d("bvrow", [1, E], F32R, kind="ExternalInput").ap(),
        d("woutT", [E, E], F32R, kind="ExternalInput").ap(),
        d("bout", [128, E // 128], F32, kind="ExternalInput").ap(),
        d("mask01", [128, 128], BF16, kind="ExternalInput").ap(),
        d("ones", [128, 128], F32R, kind="ExternalInput").ap(),
        d("onesbf", [128, H], BF16, kind="ExternalInput").ap(),
        d("ind33", [33, 128], F32R, kind="ExternalInput").ap(),
        d("outT", [E, T], F32, kind="ExternalOutput").ap(),
    ]
    with tile.TileContext(nc) as tc:
        _emit(nc, tc, aps)
    nc.compile()
    _CACHE["nc"] = nc
    return nc


def _ind33():
    ind = np.zeros((33, 128), np.float32)
    ind[0, 0:64] = 1.0
    ind[32, 64:128] = 1.0
    return ind


def _host_inputs(x, in_proj_w, in_proj_b, out_proj_w, out_proj_b,
                 c_proj_w, c_proj_b):
    f = np.float32
    x = np.ascontiguousarray(np.asarray(x, f))
    in_proj_w = np.asarray(in_proj_w, f)
    in_proj_b = np.asarray(in_proj_b, f)
    # Fold c_proj into out_proj: y@Wo^T+bo then @Wc^T+bc == y@(Wc Wo)^T +
    # (Wc bo + bc). One matmul chain on device instead of two.
    wo = np.asarray(out_proj_w, f)
    wc = np.asarray(c_proj_w, f)
    wcomb = wc @ wo
    bcomb = wc @ np.asarray(out_proj_b, f) + np.asarray(c_proj_b, f)
    shared = {
        "wqkT": np.ascontiguousarray(in_proj_w[:JQK].T).astype(ml_dtypes.bfloat16),
        "wvT": np.ascontiguousarray(in_proj_w[JQK:].T).astype(ml_dtypes.bfloat16),
        "bqk": np.ascontiguousarray(in_proj_b[:JQK].reshape(JQK // 128, 128).T),
        "bvrow": np.ascontiguousarray(in_proj_b[JQK:].reshape(1, E)),
        "woutT": np.ascontiguousarray(wcomb.T),
        "bout": np.ascontiguousarray(bcomb.reshape(E // 128, 128).T),
        "mask01": np.where(np.arange(128)[None, :] >= np.arange(128)[:, None],
                           f(1.0), f(0.0)).astype(ml_dtypes.bfloat16),
        "ones": np.ones((128, 128), f),
        "onesbf": np.ones((128, H), ml_dtypes.bfloat16),
        "ind33": _ind33(),
    }
    return [{**shared, "xT": np.ascontiguousarray(x[b].T).astype(ml_dtypes.bfloat16)}
            for b in range(B)]


def kernel(x, in_proj_w, in_proj_b, out_proj_w, out_proj_b, c_proj_w,
           c_proj_b):
    nc = _build()
    in_maps = _host_inputs(x, in_proj_w, in_proj_b, out_proj_w, out_proj_b,
                           c_proj_w, c_proj_b)
    res = run_bass_kernel_spmd(nc, in_maps, core_ids=list(range(B)),
                               trace=TRACE)
    _CACHE["last_result"] = res
    out = np.stack([res.results[b]["outT"].T for b in range(B)])
    return np.ascontiguousarray(out, dtype=np.float32)


# revision 55
# speedup vs baseline: 1.0352x; 1.0352x over previous
"""Causal self-attention block (qkv proj + 16-head causal attention + out_proj
folded with c_proj) on 8 trn2 NeuronCores, data-parallel over the batch (B=8:
one batch element per core).

Layout strategy (per core, batch element b):
  - Activations are kept feature-major [feature, token] on chip so every
    linear layer is a plain   out = W_T.T @ act   matmul chain with the
    (host-pre-transposed) weight as the stationary operand. No on-device
    transposes at all.
  - out_proj and c_proj are two back-to-back linears with no nonlinearity, so
    they are folded on the host into one matmul: W = (Wc Wo)^T, b = Wc bo+bc.
  - Attention computes transposed scores  sT[tk, tq] = k_h.T q_h  per head
    pair (row-tiled K=64 matmuls), exp with no max-subtraction (scores here
    are bounded by a few units), causal mask applied to the exp'd diagonal
    block on DVE, and the AV product consumes sT directly with token-major V
    tiles as the stationary operand. A fused ones-row in the V operand (M=65)
    yields the softmax denominator for free. Normalization is pipelined
    per head-pair: reciprocal_approx_fast straight off the psum denominator
    row, a DMA bounce through DRAM to broadcast 1/denom across partitions,
    and an in-place multiply — so the final out_proj chunk starts ~3us after
    the last AV matmul instead of waiting on a batched normalize.
  - All matmuls run in float32r (TF32-like, ~1e-4 rel precision, 4x the
    throughput of fp32 on the PE).
  - Phases are software-pipelined so the in-order PE queue always has dense
    matmul work while ACT paces the attention exps:
      S1 : qk features pairs 0-3 + V heads 0-7 (x/w DMAs interleaved;
           consts deferred; ACT exp table preloaded via a warmup exp)
      S2a: attention pairs 0-3 (both chunks) over qk pairs 4-7 + V heads
           8-15 (token tiles 0-3)
      S2b: attention chunk-0 pairs 4-7 over V heads 8-15 (token tiles 4-7)
      S3 : attention chunk-1 pairs 4-7 over out_proj chunk 0
      S4 : out_proj chunk 1 (4-deep psum pipeline)
"""

import sys

if "/opt/trn_rl_repo" not in sys.path:
    sys.path.insert(0, "/opt/trn_rl_repo")

import ml_dtypes
import numpy as np

import concourse.bass as bass
import concourse.tile as tile
from concourse import bacc, mybir
from concourse.bass_utils import run_bass_kernel_spmd

B, T, E, H = 8, 1024, 1024, 16
DH = E // H          # 64
JQK = 2 * E          # q+k fused feature dim (2048)
F32 = mybir.dt.float32
F32R = mybir.dt.float32r
BF16 = mybir.dt.bfloat16
Act = mybir.ActivationFunctionType

TRACE = False        # test harness flips this for profiled runs
PHASE_LIMIT = 4      # debug: 1=qk proj, 2=+v, 3=+attention, 4=full
_CACHE = {}


def _emit(nc, tc, aps):
    (xT, wqkT, wvT, bqk, bvrow, woutT, bout, mask01, ones,
     onesbf, ind33, outT) = aps
    ET = E // 128     # 8  e-tiles (contraction)
    TT = T // 128     # 8  token tiles
    NT = T // 512     # 2  512-wide token column chunks

    consts = tc.alloc_tile_pool(name="consts", bufs=1)
    onesb = consts.tile([128, 128], F32R, tag="onesb")
    mask01b = consts.tile([128, 128], BF16, tag="mask01b")
    bqkb = consts.tile([128, JQK // 128], F32, tag="bqkb")
    bvb = consts.tile([1, E], F32R, tag="bvb")
    boutb = consts.tile([128, E // 128], F32, tag="boutb")
    ind33b = consts.tile([33, 128], F32R, tag="ind33b")
    warm = consts.tile([1, 8], BF16, tag="warm")

    def issue_consts():
        # Deferred until the first w/x DMAs are in flight so the PE can start
        # ~4us earlier; bqkb leads because the first qk IDENTITY needs it.
        nc.sync.dma_start(out=bqkb, in_=bqk)
        nc.gpsimd.dma_start(out=mask01b, in_=mask01)
        nc.gpsimd.dma_start(out=onesb, in_=ones)
        nc.gpsimd.dma_start(out=bvb[0:1, :], in_=bvrow)
        nc.gpsimd.dma_start(out=boutb, in_=bout)
        nc.gpsimd.dma_start(out=ind33b, in_=ind33)
        # Warmup exp: pulls the ACT exp-table load off the first real
        # attention exp's critical path (fires during dense S1).
        nc.scalar.activation(out=warm, in_=mask01b[0:1, 0:8], func=Act.Exp,
                             scale=1.0 / 8.0)

    psum_mm = tc.alloc_tile_pool(name="psum_mm", bufs=1, space="PSUM")
    psum_att = tc.alloc_tile_pool(name="psum_att", bufs=1, space="PSUM")
    p_dram = tc.alloc_tile_pool(name="p_dram", bufs=1, space="DRAM")
    p_y = tc.alloc_tile_pool(name="p_y", bufs=1)
    p_qk = tc.alloc_tile_pool(name="p_qk", bufs=1)
    p_v = tc.alloc_tile_pool(name="p_v", bufs=1)
    p_w3 = tc.alloc_tile_pool(name="p_w3", bufs=16)
    p_x = tc.alloc_tile_pool(name="p_x", bufs=1)
    p_wqk = tc.alloc_tile_pool(name="p_wqk", bufs=24)
    dden = p_dram.tile([64, 512], F32, tag="dden")
    yt = p_y.tile([128, ET, T], F32R)
    qkt = p_qk.tile([128, JQK // 128, T], F32R)
    vt = p_v.tile([128, TT, H, DH + 1], BF16)
    xt = p_x.tile([128, ET, T], BF16)

    def mm_psum(tag="mm", pool=None, bufs=2):
        return (pool or psum_mm).tile([128, 512], F32, tag=tag, bufs=bufs,
                                      name="ps_" + tag)

    # ---- dense generators: qkv projection ---------------------------------
    def qk_gen(jg, after=None):
        """qkT[j, t] = Wqk x^T + bqk for the 512-wide feature group jg."""
        wtiles = []
        for et in range(ET):
            wt = p_wqk.tile([128, 512], BF16, tag="wqk", name="wt")
            # first weight group split across the sync and gpsimd rings so
            # the DMA-latency-bound startup streams two tiles at a time
            ring = nc.gpsimd if (jg == 0 and et % 2 == 1) else nc.sync
            ring.dma_start(out=wt, in_=wqkT[et * 128:(et + 1) * 128,
                                            jg * 512:(jg + 1) * 512])
            wtiles.append(wt)
            if jg == 0:                    # x loads ride the ACT DGE ring —
                nc.scalar.dma_start(       # startup streams w (sync) and x
                    out=xt[:, et, 0:512],  # (scalar) in parallel
                    in_=xT[et * 128:(et + 1) * 128, 0:512])
        if after is not None:
            after()
        if jg == 0:
            for et in range(ET):
                nc.gpsimd.dma_start(out=xt[:, et, 512:1024],
                                    in_=xT[et * 128:(et + 1) * 128,
                                           512:1024])
        for th in range(NT):
            for js in range(4):
                jt = jg * 4 + js
                ps = mm_psum()
                for et in range(ET):
                    nc.tensor.matmul(
                        ps,
                        wtiles[et][:, js * 128:(js + 1) * 128],
                        xt[:, et, th * 512:(th + 1) * 512],
                        start=(et == 0), stop=(et == ET - 1))
                    yield
                nc.scalar.activation(
                    out=qkt[:, jt, th * 512:(th + 1) * 512], in_=ps,
                    func=Act.Identity, bias=bqkb[:, jt:jt + 1], scale=1.0)

    wv_cache = {}

    def vb_gen(jh, tt0, tt1):
        """v[t, h, d] token-major for heads 8*jh..8*jh+7 (+bias via ones-row
        matmul), with a bf16 ones column at d=64 for the fused denominator.
        Emits token tiles tt0..tt1 (split so S2b keeps dense filler)."""
        if jh == 0 and tt0 == 0:
            for tt in range(TT):
                nc.gpsimd.dma_start(out=vt[:, tt, :, DH], in_=onesbf)
        if jh in wv_cache:
            wvtiles = wv_cache[jh]
        else:
            wvtiles = []
            for et in range(ET):
                wt = p_wqk.tile([128, 512], BF16, tag="wqk", name="wt")
                nc.sync.dma_start(out=wt, in_=wvT[et * 128:(et + 1) * 128,
                                                 jh * 512:(jh + 1) * 512])
                wvtiles.append(wt)
            wv_cache[jh] = wvtiles
        for tt in range(tt0, tt1):
            ps = mm_psum()
            for et in range(ET):
                nc.tensor.matmul(
                    ps,
                    xt[:, et, tt * 128:(tt + 1) * 128],
                    wvtiles[et],
                    start=(et == 0), stop=False)
                yield
            nc.tensor.matmul(
                ps, onesb[0:1, 0:128], bvb[0:1, jh * 512:(jh + 1) * 512],
                start=False, stop=True)
            yield
            # the last V drains land amid the S2b drain chains on DVE; route
            # them to ACT (Identity, NOT Copy — Copy swaps the act table and
            # forces a 1.3us reload around every exp)
            if jh == 1 and tt >= 6:
                nc.scalar.activation(
                    out=vt[:, tt, jh * 8:(jh + 1) * 8, 0:DH],
                    in_=ps.rearrange("p (h d) -> p h d", d=DH),
                    func=Act.Identity, bias=0.0, scale=1.0)
            else:
                nc.vector.tensor_copy(
                    out=vt[:, tt, jh * 8:(jh + 1) * 8, 0:DH],
                    in_=ps.rearrange("p (h d) -> p h d", d=DH))

    # ---- attention generator (yields once per tk-iteration) ---------------
    LAG = 3

    def att_gen(c, a, p_esc, p_nrm):
        cs = c * 512
        last_it = 4 * c + 3
        qj = a                             # q tile of the pair
        kj = (JQK // 2) // 128 + a         # k tile of the pair
        avps = [psum_att.tile([128, 512], F32, tag=f"av{p}", bufs=1,
                              name=f"avp{p}") for p in range(2)]
        pend = []

        def emit_av(it, sub, clen, esc):
            for p in range(2):
                nc.tensor.matmul(
                    avps[p][0:DH + 1, sub:sub + clen],
                    vt[:, it, 2 * a + p, :],
                    esc[:, p, :clen],
                    start=(it == 0), stop=(it == last_it),
                    skip_group_check=True)

        for it in range(last_it + 1):
            n0 = it * 128
            lo = max(n0, cs)
            sub = lo - cs
            clen = 512 - sub
            scp = psum_att.tile([128, 2, 512], F32, tag="sc", bufs=2,
                                name="scp")
            for p in range(2):             # paired heads: row-tiled matmuls
                pb = p * 64
                nc.tensor.matmul(
                    scp[:, p, :clen],
                    qkt[pb:pb + 64, kj, n0:n0 + 128],
                    qkt[pb:pb + 64, qj, lo:lo + clen],
                    start=True, stop=True)
            esc = p_esc.tile([128, 2, 512], BF16, tag="esc", name="esc")
            nc.scalar.activation(out=esc[:, :, :clen], in_=scp[:, :, :clen],
                                 func=Act.Exp, scale=1.0 / 8.0)
            if n0 >= cs:                   # diagonal block: causal mask,
                nc.vector.tensor_mul(      # off the PE chain thanks to LAG
                    esc[:, :, 0:128], esc[:, :, 0:128],
                    mask01b[:, None, :].broadcast_to([128, 2, 128]))
            pend.append((it, sub, clen, esc))
            if len(pend) > LAG:
                emit_av(*pend.pop(0))
            yield
        for args in pend:
            emit_av(*args)
        for p in range(2):                 # drain unnormalized y
            nc.vector.tensor_copy(out=yt[p * 64:p * 64 + 64, qj,
                                         cs:cs + 512],
                                  in_=avps[p][0:DH, :])
        # Per-pair normalization, pipelined behind the remaining attention
        # units: reciprocal straight off the psum denominator rows (partition
        # bases 64/96 are engine-addressable), one DMA out to DRAM, a
        # partition-broadcast DMA back, and an in-place multiply.
        # custom-DVE ops only honor partition offset 0, so stage the den rows
        # to partitions 0/32 of a scratch tile and recip in place.
        rst = [p_nrm.tile([128, 512], F32, tag="rstg", bufs=2, name="rstg")
               for _ in range(2)]
        r = 32 * c + 2 * a
        for p in range(2):
            nc.vector.tensor_copy(out=rst[p][0:1, :],
                                  in_=avps[p][DH:DH + 1, :])
            nc.vector.reciprocal_approx_fast(out=rst[p][0:1, :],
                                             in_=rst[p][0:1, :])
        for p in range(2):
            nc.sync.dma_start(out=dden[r + p:r + p + 1, :],
                              in_=rst[p][0:1, :])
        rb = p_nrm.tile([128, 512], F32, tag="rb", bufs=2, name="rb")
        for p in range(2):
            row = dden[r + p:r + p + 1, :]
            srcb = bass.AP(tensor=row.tensor, offset=row.offset,
                           ap=[[0, 64]] + list(row.ap)[1:])
            nc.sync.dma_start(out=rb[p * 64:(p + 1) * 64, :], in_=srcb)
        nc.vector.tensor_mul(yt[:, a, cs:cs + 512], yt[:, a, cs:cs + 512],
                             rb)

    # ---- drivers ----------------------------------------------------------
    def run_dense(dense, n=None):
        steps = 0
        while dense and (n is None or steps < n):
            try:
                next(dense[0])
                steps += 1
            except StopIteration:
                dense.pop(0)
        return steps

    def drive(att_units, dense, ratio=5):
        att_units = list(att_units)
        while att_units:
            try:
                next(att_units[0])
            except StopIteration:
                att_units.pop(0)
                continue
            run_dense(dense, ratio)
        run_dense(dense)

    # S1: dense deps for attention pairs 0-3
    dense1 = [qk_gen(0, after=issue_consts), qk_gen(2)]
    if PHASE_LIMIT >= 2:
        dense1.append(vb_gen(0, 0, TT))
    run_dense(dense1)

    # S2a: attention pairs 0-3 (both chunks) over qk pairs 4-7 + V tt 0-3
    p_esc1 = tc.alloc_tile_pool(name="p_esc1", bufs=4)
    p_nrm1 = tc.alloc_tile_pool(name="p_nrm1", bufs=1)
    dense2 = [qk_gen(1), qk_gen(3)]
    if PHASE_LIMIT >= 2:
        dense2.append(vb_gen(1, 0, 4))
    att2 = [att_gen(c, a, p_esc1, p_nrm1)
            for a in range(4) for c in range(NT)] if PHASE_LIMIT >= 3 else []
    drive(att2, dense2, ratio=4)

    # S2b: attention chunk-0 pairs 4-7 and chunk-1 pairs 4-5 interleaved over
    # the remaining V token tiles. The chunk-1 units give the PE independent
    # score/exp work while the chunk-0 drain + normalize chains serialize on
    # DVE and the DMA queue (the pipeline tail would otherwise idle ~10us).
    # prefetch the out_proj weights now — their DMAs have no deps and would
    # otherwise monopolize the queue exactly when S3's first groups need them
    wout_tiles = []
    if PHASE_LIMIT >= 4:
        for og in range(2):
            for et in range(ET):
                wt = p_w3.tile([128, 512], F32R, tag="w3", name="wt3")
                nc.sync.dma_start(
                    out=wt, in_=woutT[et * 128:(et + 1) * 128,
                                      og * 512:(og + 1) * 512])
                wout_tiles.append(wt)
    dense2b = [vb_gen(1, 4, TT)] if PHASE_LIMIT >= 2 else []
    att2b = [att_gen(c, a, p_esc1, p_nrm1)
             for c, a in [(0, 4), (0, 5), (1, 4), (0, 6), (1, 5), (0, 7)]
             ] if PHASE_LIMIT >= 3 else []
    drive(att2b, dense2b, ratio=3)
    p_nrm1.release()
    p_esc1.release()
    p_wqk.release()
    p_x.release()

    # S3: attention chunk-1 pairs 6-7 over out_proj chunk 0
    p_out = tc.alloc_tile_pool(name="p_out", bufs=3)
    p_esc2 = tc.alloc_tile_pool(name="p_esc2", bufs=4)
    p_nrm2 = tc.alloc_tile_pool(name="p_nrm2", bufs=1)

    def oproj_gen(th, pool, tag, bufs):
        for og in range(2):
            for os_ in range(4):
                ot = og * 4 + os_
                ps = mm_psum(tag=tag, pool=pool, bufs=bufs)
                for et in range(ET):
                    nc.tensor.matmul(
                        ps,
                        wout_tiles[og * ET + et][:, os_ * 128:(os_ + 1) * 128],
                        yt[:, et, th * 512:(th + 1) * 512],
                        start=(et == 0), stop=(et == ET - 1))
                    yield
                ob = p_out.tile([128, 512], F32, tag="ob", name="ob")
                nc.scalar.activation(out=ob, in_=ps, func=Act.Identity,
                                     bias=boutb[:, ot:ot + 1], scale=1.0)
                nc.sync.dma_start(
                    out=outT[ot * 128:(ot + 1) * 128,
                             th * 512:(th + 1) * 512],
                    in_=ob)

    att3 = [att_gen(1, a, p_esc2, p_nrm2)
            for a in range(6, 8)] if PHASE_LIMIT >= 3 else []
    dense3 = [oproj_gen(0, psum_mm, "mm", 2)] if PHASE_LIMIT >= 4 else []
    drive(att3, dense3, ratio=3)

    # S4: out_proj chunk 1 on a 4-deep psum pipeline (attention psums freed)
    psum_att.release()
    if PHASE_LIMIT >= 4:
        psum_tail = tc.alloc_tile_pool(name="psum_tail", bufs=1, space="PSUM")
        run_dense([oproj_gen(1, psum_tail, "mmt", 4)])
        psum_tail.release()
    p_nrm2.release()
    p_esc2.release()
    p_out.release()
    p_w3.release()
    p_v.release()
    p_qk.release()
    p_y.release()
    p_dram.release()
    psum_mm.release()
    consts.release()


def _build():
    if "nc" in _CACHE:
        return _CACHE["nc"]
    nc = bacc.Bacc("TRN2", target_bir_lowering=False, debug=False,
                   enable_asserts=True, num_devices=8)
    d = nc.dram_tensor
    aps = [
        d("xT", [E, T], BF16, kind="ExternalInput").ap(),
        d("wqkT", [E, JQK], BF16, kind="ExternalInput").ap(),
        d("wvT", [E, E], BF16, kind="ExternalInput").ap(),
        d("bqk", [128, JQK // 128], F32, kind="ExternalInput").ap(),
        d("bvrow", [1, E], F32R, kind="ExternalInput").ap(),
        d("woutT", [E, E], F32R, kind="ExternalInput").ap(),
        d("bout", [128, E // 128], F32, kind="ExternalInput").ap(),
        d("mask01", [128, 128], BF16, kind="ExternalInput").ap(),
        d("ones", [128, 128], F32R, kind="ExternalInput").ap(),
        d("onesbf", [128, H], BF16, kind="ExternalInput").ap(),
        d("ind33", [33, 128], F32R, kind="ExternalInput").ap(),
        d("outT", [E, T], F32, kind="ExternalOutput").ap(),
    ]
    with tile.TileContext(nc) as tc:
        _emit(nc, tc, aps)
    nc.compile()
    _CACHE["nc"] = nc
    return nc


def _ind33():
    ind = np.zeros((33, 128), np.float32)
    ind[0, 0:64] = 1.0
    ind[32, 64:128] = 1.0
    return ind


def _host_inputs(x, in_proj_w, in_proj_b, out_proj_w, out_proj_b,
                 c_proj_w, c_proj_b):
    f = np.float32
    x = np.ascontiguousarray(np.asarray(x, f))
    in_proj_w = np.asarray(in_proj_w, f)
    in_proj_b = np.asarray(in_proj_b, f)
    # Fold c_proj into out_proj: y@Wo^T+bo then @Wc^T+bc == y@(Wc Wo)^T +
    # (Wc bo + bc). One matmul chain on device instead of two.
    wo = np.asarray(out_proj_w, f)
    wc = np.asarray(c_proj_w, f)
    wcomb = wc @ wo
    bcomb = wc @ np.asarray(out_proj_b, f) + np.asarray(c_proj_b, f)
    shared = {
        "wqkT": np.ascontiguousarray(in_proj_w[:JQK].T).astype(ml_dtypes.bfloat16),
        "wvT": np.ascontiguousarray(in_proj_w[JQK:].T).astype(ml_dtypes.bfloat16),
        "bqk": np.ascontiguousarray(in_proj_b[:JQK].reshape(JQK // 128, 128).T),
        "bvrow": np.ascontiguousarray(in_proj_b[JQK:].reshape(1, E)),
        "woutT": np.ascontiguousarray(wcomb.T),
        "bout": np.ascontiguousarray(bcomb.reshape(E // 128, 128).T),
        "mask01": np.where(np.arange(128)[None, :] >= np.arange(128)[:, None],
                           f(1.0), f(0.0)).astype(ml_dtypes.bfloat16),
        "ones": np.ones((128, 128), f),
        "onesbf": np.ones((128, H), ml_dtypes.bfloat16),
        "ind33": _ind33(),
    }
    return [{**shared, "xT": np.ascontiguousarray(x[b].T).astype(ml_dtypes.bfloat16)}
            for b in range(B)]


def kernel(x, in_proj_w, in_proj_b, out_proj_w, out_proj_b, c_proj_w,
           c_proj_b):
    nc = _build()
    in_maps = _host_inputs(x, in_proj_w, in_proj_b, out_proj_w, out_proj_b,
                           c_proj_w, c_proj_b)
    res = run_bass_kernel_spmd(nc, in_maps, core_ids=list(range(B)),
                               trace=TRACE)
    _CACHE["last_result"] = res
    out = np.stack([res.results[b]["outT"].T for b in range(B)])
    return np.ascontiguousarray(out, dtype=np.float32)
